# revision 1
# baseline (speedup 1.0000x reference)
"""Trainium2 Bass kernel for nn_Llama_head (paired two-tower MLP head).

Computes sigmoid(rowwise_dot(mlp_u(xu), mlp_i(xv))) for N=32768 rows,
data-parallel across 8 NeuronCores (N sharded, weights replicated).

Host-side prep (per core shard, alongside the weight bf16 cast/packing):
x is cast to bf16 and laid out d-major as [p=128, blk=8, k=32, n=512]
(element (p,b,k,n) = x[b*512+n, k*128+p]).  This kills the two baseline
PE costs that dominated the old trace: the 128x128 PE transposes (a
third of all PE FLOPs) and their PSUM->SBUF DVE copies, and it halves
HBM traffic (67 MB bf16 vs 134 MB f32 per core).

Per-core dataflow (Nc = 4096 rows, blocks of NB = 512 rows):
  1. HWDGE DMA: xT block [128, 32, 512] bf16, 32KB contiguous per
     partition (line rate).
  2. Layer 1 directly: hT[h, n] += w1[d, h].T @ xT[d, n], 32 k-tiles
     accumulated in PSUM (f32); moving operand is a slice of the DMA'd
     tile - no transpose, no copy.
  3. ACT: h = relu(hT + b1) PSUM->SBUF (bf16), bias per-partition.
  4. Layer 2: uT[64, n] = w2.T @ h (2 k-tiles of 128).
  5. DVE: u = uT + b2; prod = u * v; PE: ones.T @ prod -> diag[1, n];
     ACT: sigmoid -> s_blk; DMA s_blk -> out (scalar HWDGE queue).
The block tail (layer 2 / diag / out) for block b is emitted in the
middle of block b+1's layer-1 stream so the PE never waits on ACT/DVE.
"""

import os

import numpy as np
import ml_dtypes

# Problem shape (hardcoded per harness contract).
N_FULL = 32768
D = 4096
H = 256
O = 64
N_CORES = 8

NC_ROWS = N_FULL // N_CORES  # rows per core
NB = 512                     # rows per block
TRACE = bool(int(os.environ.get("KERNEL_TRACE", "0")))

LAST_RESULTS = None  # BassKernelResults of the most recent run (for profiling)

_PROGRAM = None


def _build_program():
    from contextlib import ExitStack

    import concourse.mybir as mybir
    import concourse.tile as tile
    from concourse import bacc

    f32 = mybir.dt.float32
    bf16 = mybir.dt.bfloat16
    AF = mybir.ActivationFunctionType

    n_rows = NC_ROWS
    nb = NB
    nblk = n_rows // nb
    kt = D // 128
    hh_t = H // 128

    nc = bacc.Bacc("TRN2")

    # x is pre-transposed/pre-cast on host: [p, blk, k, n]. Weights are
    # host-packed p-major too, so every DMA is >=16KB-contiguous per
    # partition (~128 descriptors; the naive (k p) h AP costs 4096).
    xu = nc.dram_tensor("xu", [128, nblk, kt, nb], bf16, kind="ExternalInput")
    xv = nc.dram_tensor("xv", [128, nblk, kt, nb], bf16, kind="ExternalInput")
    w1u = nc.dram_tensor("w1u", [128, kt, H], bf16, kind="ExternalInput")
    w1i = nc.dram_tensor("w1i", [128, kt, H], bf16, kind="ExternalInput")
    w2u = nc.dram_tensor("w2u", [128, hh_t, O], bf16, kind="ExternalInput")
    w2i = nc.dram_tensor("w2i", [128, hh_t, O], bf16, kind="ExternalInput")
    # Packed small constants: biases f32 [128, 6]; ones column bf16.
    cst_d = nc.dram_tensor("cst", [128, 6], f32, kind="ExternalInput")
    ones_d = nc.dram_tensor("ones", [O, 1], bf16, kind="ExternalInput")
    out = nc.dram_tensor("out", [n_rows], f32, kind="ExternalOutput")

    x_d = {"u": xu, "i": xv}

    with ExitStack() as ctx:
        tc = ctx.enter_context(tile.TileContext(nc))

        wpool = ctx.enter_context(tc.tile_pool(name="weights", bufs=1))
        xtp = ctx.enter_context(tc.tile_pool(name="xt", bufs=4))
        hp = ctx.enter_context(tc.tile_pool(name="h", bufs=8))
        uvp = ctx.enter_context(tc.tile_pool(name="uv", bufs=6))
        sp = ctx.enter_context(tc.tile_pool(name="sacc", bufs=2))
        ps_h = ctx.enter_context(tc.tile_pool(name="psh", bufs=4, space="PSUM"))
        ps_uv = ctx.enter_context(tc.tile_pool(name="psuv", bufs=2, space="PSUM"))
        ps_d = ctx.enter_context(tc.tile_pool(name="psd", bufs=2, space="PSUM"))

        # --- constants ride the scalar HWDGE queue so the sync queue's first
        # bytes are w1/x; w1/x DMAs are emitted inside the b==0 loop,
        # chunk-interleaved in PE consumption order on the FIFO sync queue ---
        cst = wpool.tile([128, 6], f32, tag="cst", name="cst")
        nc.scalar.dma_start(cst, cst_d[:])
        ones_sb = wpool.tile([O, 1], bf16, tag="ones", name="ones")
        nc.scalar.dma_start(ones_sb, ones_d[:])
        b1_sb = {"u": cst[:, 0:2], "i": cst[:, 2:4]}
        b2_sb = {"u": cst[:O, 4:5], "i": cst[:O, 5:6]}

        w1_d = {"u": w1u, "i": w1i}
        w2_d = {"u": w2u, "i": w2i}
        w1_sb = {}
        w2_sb = {}
        for name in ("u", "i"):
            w1_sb[name] = wpool.tile([128, kt, H], bf16, tag=f"w1{name}", name=f"w1{name}")
            w2_sb[name] = wpool.tile([128, hh_t, O], bf16, tag=f"w2{name}", name=f"w2{name}")

        def emit_l1(sname, xt, w, n0=0):
            """Layer-1 matmuls for one tower of one block; relu to SBUF."""
            ph = [ps_h.tile([128, w], f32, tag="ph", name="ph") for _ in range(hh_t)]
            for k in range(kt):
                mv = xt[:, k, n0 : n0 + w]
                for hh in range(hh_t):
                    nc.tensor.matmul(
                        ph[hh],
                        w1_sb[sname][:, k, hh * 128 : (hh + 1) * 128],
                        mv,
                        start=(k == 0),
                        stop=(k == kt - 1),
                    )
            hsb = [hp.tile([128, w], bf16, tag="h", name="h") for _ in range(hh_t)]
            for hh in range(hh_t):
                nc.scalar.activation(
                    hsb[hh], ph[hh], AF.Relu, bias=b1_sb[sname][:, hh : hh + 1]
                )
            return hsb

        def emit_tail(r0, hs, w):
            """Layer 2 + rowwise dot + sigmoid + DMA for rows [r0, r0+w)."""
            stash = {}
            for sname in ("u", "i"):
                puv = ps_uv.tile([O, w], f32, tag="puv", name="puv")
                for hh in range(hh_t):
                    nc.tensor.matmul(
                        puv,
                        w2_sb[sname][:, hh, :],
                        hs[sname][hh],
                        start=(hh == 0),
                        stop=(hh == hh_t - 1),
                    )
                usb = uvp.tile([O, w], bf16, tag="uv", name="uv")
                nc.vector.tensor_scalar_add(usb, puv, b2_sb[sname])
                stash[sname] = usb
            prod = uvp.tile([O, w], bf16, tag="prod", name="prod")
            nc.vector.tensor_mul(prod, stash["u"], stash["i"])
            ps = ps_d.tile([1, w], f32, tag="psd", name="psd")
            nc.tensor.matmul(ps, ones_sb, prod, start=True, stop=True)
            s_blk = sp.tile([1, w], f32, tag="sblk", name="s_blk")
            nc.scalar.activation(s_blk, ps, AF.Sigmoid)
            nc.scalar.dma_start(out[r0 : r0 + w], s_blk)

        # All x/w transfers ride the single sync HWDGE ring in consumption
        # order. Measured twice: splitting across the two HWDGE rings
        # (by tower or by alternating chunks) only desynchronizes delivery
        # and adds 15-20us of PE stalls.
        def xdma(dst, src):
            nc.sync.dma_start(dst, src)

        # --- main loop; tail(b-1) is emitted between block b's two towers ---
        pending = None  # (block_idx, {"u": hsb, "i": hsb})
        for b in range(nblk):
            xts = {}
            for sname in ("u", "i"):
                xt = xtp.tile([128, kt, nb], bf16, tag="xt", name="xt")
                if b == 0:
                    # Interleave w1 and x chunks in PE consumption order
                    # (small leading chunk) so the first matmuls start ~12us
                    # in and stay mostly fed.
                    for q0, q1 in ((0, 4), (4, 12), (12, 20), (20, 28), (28, 32)):
                        xdma(w1_sb[sname][:, q0:q1, :], w1_d[sname][:, q0:q1, :])
                        xdma(xt[:, q0:q1, :], x_d[sname][:, b, q0:q1, :])
                else:
                    # Halves: the first 16 k-tiles become consumable without
                    # waiting for the whole 4 MB block to land.
                    half = kt // 2
                    xdma(xt[:, :half, :], x_d[sname][:, b, :half, :])
                    xdma(xt[:, half:, :], x_d[sname][:, b, half:, :])
                xts[sname] = xt
            if b == 0:
                for sname in ("u", "i"):
                    xdma(w2_sb[sname], w2_d[sname][:])

            if b < nblk - 1:
                hs = {}
                hs["u"] = emit_l1("u", xts["u"], nb)
                if pending is not None:
                    emit_tail(*pending)
                hs["i"] = emit_l1("i", xts["i"], nb)
                pending = (b * nb, hs, nb)
            else:
                # Last block: pull everything that can run early out of the
                # end-of-kernel serial chain. u-tower layer 2 runs right
                # after tail(b-1); the i-tower runs h-sequentially with its
                # h0 layer-2 partial embedded mid-stream, so only
                # relu(h1) -> l2(h1) -> dot -> sigmoid remain at the end.
                hs_u = emit_l1("u", xts["u"], nb)
                if pending is not None:
                    emit_tail(*pending)
                puv_u = ps_uv.tile([O, nb], f32, tag="puv", name="puv")
                for hh in range(hh_t):
                    nc.tensor.matmul(
                        puv_u,
                        w2_sb["u"][:, hh, :],
                        hs_u[hh],
                        start=(hh == 0),
                        stop=(hh == hh_t - 1),
                    )
                usb_u = uvp.tile([O, nb], bf16, tag="uv", name="uv")
                nc.vector.tensor_scalar_add(usb_u, puv_u, b2_sb["u"])

                xt = xts["i"]
                ph = [ps_h.tile([128, nb], f32, tag="ph", name="ph") for _ in range(hh_t)]
                hs_i = [hp.tile([128, nb], bf16, tag="h", name="h") for _ in range(hh_t)]
                puv_i = ps_uv.tile([O, nb], f32, tag="puv", name="puv")
                for k in range(kt):
                    nc.tensor.matmul(
                        ph[0], w1_sb["i"][:, k, 0:128], xt[:, k, :],
                        start=(k == 0), stop=(k == kt - 1),
                    )
                nc.scalar.activation(hs_i[0], ph[0], AF.Relu, bias=b1_sb["i"][:, 0:1])
                for k in range(kt):
                    nc.tensor.matmul(
                        ph[1], w1_sb["i"][:, k, 128:256], xt[:, k, :],
                        start=(k == 0), stop=(k == kt - 1),
                    )
                    if k == 6:
                        nc.tensor.matmul(
                            puv_i, w2_sb["i"][:, 0, :], hs_i[0],
                            start=True, stop=False,
                        )
                nc.scalar.activation(hs_i[1], ph[1], AF.Relu, bias=b1_sb["i"][:, 1:2])
                nc.tensor.matmul(
                    puv_i, w2_sb["i"][:, 1, :], hs_i[1], start=False, stop=True
                )
                usb_i = uvp.tile([O, nb], bf16, tag="uv", name="uv")
                nc.vector.tensor_scalar_add(usb_i, puv_i, b2_sb["i"])
                prod = uvp.tile([O, nb], bf16, tag="prod", name="prod")
                nc.vector.tensor_mul(prod, usb_u, usb_i)
                ps = ps_d.tile([1, nb], f32, tag="psd", name="psd")
                nc.tensor.matmul(ps, ones_sb, prod, start=True, stop=True)
                s_blk = sp.tile([1, nb], f32, tag="sblk", name="s_blk")
                nc.scalar.activation(s_blk, ps, AF.Sigmoid)
                nc.scalar.dma_start(out[b * nb : (b + 1) * nb], s_blk)

    nc.compile()
    return nc


def _pack_cst(b1u, b1i, b2u, b2i):
    """[128, 6] f32: b1u as 2 cols, b1i as 2 cols, b2u, b2i (zero-padded)."""
    cst = np.zeros((128, 6), dtype=np.float32)
    cst[:, 0:2] = b1u.reshape(2, 128).T
    cst[:, 2:4] = b1i.reshape(2, 128).T
    cst[: b2u.shape[0], 4] = b2u
    cst[: b2i.shape[0], 5] = b2i
    return cst


def _pack_x(x_shard):
    """[Nc, D] f32 -> bf16 [p, blk, k, n] with (p,b,k,n) = x[b*NB+n, k*128+p]."""
    nblk = x_shard.shape[0] // NB
    kt = D // 128
    xb = x_shard.astype(ml_dtypes.bfloat16)
    return np.ascontiguousarray(
        xb.reshape(nblk, NB, kt, 128).transpose(3, 0, 2, 1)
    )


def _pack_w(w):
    """[K*128, M] f32 -> bf16 [p, k, M] with (p,k,m) = w[k*128+p, m]."""
    kt = w.shape[0] // 128
    wb = np.asarray(w, dtype=np.float32).astype(ml_dtypes.bfloat16)
    return np.ascontiguousarray(wb.reshape(kt, 128, w.shape[1]).transpose(1, 0, 2))


def _get_program():
    global _PROGRAM
    if _PROGRAM is None:
        _PROGRAM = _build_program()
    return _PROGRAM


def kernel(
    user_origin_emb,
    item_origin_emb,
    u_w1,
    u_b1,
    u_w2,
    u_b2,
    i_w1,
    i_b1,
    i_w2,
    i_b2,
):
    global LAST_RESULTS
    from concourse.bass_utils import run_bass_kernel_spmd

    xu = np.asarray(user_origin_emb, dtype=np.float32)
    xv = np.asarray(item_origin_emb, dtype=np.float32)
    ones = np.ones((O, 1), dtype=ml_dtypes.bfloat16)
    shared = {
        "w1u": _pack_w(u_w1),
        "w1i": _pack_w(i_w1),
        "w2u": _pack_w(u_w2),
        "w2i": _pack_w(i_w2),
        "cst": _pack_cst(
            np.asarray(u_b1, dtype=np.float32),
            np.asarray(i_b1, dtype=np.float32),
            np.asarray(u_b2, dtype=np.float32),
            np.asarray(i_b2, dtype=np.float32),
        ),
        "ones": ones,
    }

    nc = _get_program()
    n_rows = xu.shape[0] // N_CORES
    in_maps = [
        {
            "xu": _pack_x(xu[c * n_rows : (c + 1) * n_rows]),
            "xv": _pack_x(xv[c * n_rows : (c + 1) * n_rows]),
            **shared,
        }
        for c in range(N_CORES)
    ]
    res = run_bass_kernel_spmd(nc, in_maps, core_ids=list(range(N_CORES)), trace=TRACE)
    LAST_RESULTS = res
    return np.concatenate([r["out"] for r in res.results], axis=0)



# revision 8
# speedup vs baseline: 1.1923x; 1.1923x over previous
"""Trainium2 Bass kernel for nn_Llama_head (paired two-tower MLP head).

Computes sigmoid(rowwise_dot(mlp_u(xu), mlp_i(xv))) for N=32768 rows,
data-parallel across 8 NeuronCores (N sharded, weights replicated).

Layer 1 runs as a k-split precision hybrid: the first K8T k-tiles
(d < K8T*128) use fp8 e4m3 with DoubleRow perf mode (2 k-planes per
matmul instruction, 2x PE throughput, 1 byte/elem HBM), the remaining
k-tiles use bf16 (exact to working precision).  Pure fp8 measures
max-err 0.0258 against the f32 reference (gate is 2e-2); the 16/16
split measures 0.0193.  Quantization: x -> e4m3 directly, w1 ->
e4m3(256*w1) with the 1/256 descale folded into the ReLU activation
(relu is positively homogeneous).  Layer 2 + rowwise dot stay bf16.

Scheduling (the baseline lost ~560ns twice per block to a false
cross-engine dependency): the tail (layer 2 of block b-1) is emitted
BEFORE block b's ReLU ACTs so its sem wait doesn't include them, and
the diag matmul + sigmoid are deferred ~10 matmuls into the i-tower
stream so the DVE chain they depend on has completed.

Host-side prep (per core shard): x split d-wise into an fp8 tensor
[p=128, blk, K8T, n=512] and a bf16 tensor [p, blk, KBT, n]
(element (p,b,k,n) = x[b*512+n, k*128+p]); every DMA is >=4KB
contiguous per partition.
"""

import os

import numpy as np
import ml_dtypes

# Problem shape (hardcoded per harness contract).
N_FULL = 32768
D = 4096
H = 256
O = 64
N_CORES = 8

NC_ROWS = N_FULL // N_CORES  # rows per core
NB = 512                     # rows per block
KT = D // 128                # 32 k-tiles
K8T = 12                     # k-tiles in fp8 (must be even); rest bf16
KBT = KT - K8T
SW = 256.0                   # w1 fp8 pre-scale (power of 2; undone in ACT)
TRACE = bool(int(os.environ.get("KERNEL_TRACE", "0")))

LAST_RESULTS = None  # BassKernelResults of the most recent run (for profiling)

_PROGRAM = None


def _build_program():
    from contextlib import ExitStack

    import concourse.mybir as mybir
    import concourse.tile as tile
    from concourse import bacc

    f32 = mybir.dt.float32
    bf16 = mybir.dt.bfloat16
    fp16 = mybir.dt.float16
    fp8 = mybir.dt.float8e4
    AF = mybir.ActivationFunctionType
    DR = mybir.MatmulPerfMode.DoubleRow

    n_rows = NC_ROWS
    nb = NB
    nblk = n_rows // nb
    hh_t = H // 128

    nc = bacc.Bacc("TRN2")

    x8u = nc.dram_tensor("x8u", [128, nblk, K8T, nb], fp8, kind="ExternalInput")
    x8v = nc.dram_tensor("x8v", [128, nblk, K8T, nb], fp8, kind="ExternalInput")
    xbu = nc.dram_tensor("xbu", [128, nblk, KBT, nb], bf16, kind="ExternalInput")
    xbv = nc.dram_tensor("xbv", [128, nblk, KBT, nb], bf16, kind="ExternalInput")
    w18u = nc.dram_tensor("w18u", [128, K8T, H], fp8, kind="ExternalInput")
    w18i = nc.dram_tensor("w18i", [128, K8T, H], fp8, kind="ExternalInput")
    w1bu = nc.dram_tensor("w1bu", [128, KBT, H], bf16, kind="ExternalInput")
    w1bi = nc.dram_tensor("w1bi", [128, KBT, H], bf16, kind="ExternalInput")
    w2u = nc.dram_tensor("w2u", [128, hh_t, O], fp16, kind="ExternalInput")
    w2i = nc.dram_tensor("w2i", [128, hh_t, O], fp16, kind="ExternalInput")
    # Packed small constants: biases f32 [128, 6]; ones column bf16.
    cst_d = nc.dram_tensor("cst", [128, 6], f32, kind="ExternalInput")
    ones_d = nc.dram_tensor("ones", [O, 1], fp16, kind="ExternalInput")
    out = nc.dram_tensor("out", [n_rows], f32, kind="ExternalOutput")

    x8_d = {"u": x8u, "i": x8v}
    xb_d = {"u": xbu, "i": xbv}
    w18_d = {"u": w18u, "i": w18i}
    w1b_d = {"u": w1bu, "i": w1bi}
    w2_d = {"u": w2u, "i": w2i}

    with ExitStack() as ctx:
        tc = ctx.enter_context(tile.TileContext(nc))

        wpool = ctx.enter_context(tc.tile_pool(name="weights", bufs=1))
        x8p = ctx.enter_context(tc.tile_pool(name="x8", bufs=4))
        xbp = ctx.enter_context(tc.tile_pool(name="xb", bufs=4))
        hp = ctx.enter_context(tc.tile_pool(name="h", bufs=8))
        uvp = ctx.enter_context(tc.tile_pool(name="uv", bufs=6))
        sp = ctx.enter_context(tc.tile_pool(name="sacc", bufs=2))
        ps_h = ctx.enter_context(tc.tile_pool(name="psh", bufs=4, space="PSUM"))
        ps_uv = ctx.enter_context(tc.tile_pool(name="psuv", bufs=2, space="PSUM"))
        ps_d = ctx.enter_context(tc.tile_pool(name="psd", bufs=2, space="PSUM"))

        # Constants ride the scalar HWDGE queue so the sync queue's first
        # bytes are w1/x.
        cst = wpool.tile([128, 6], f32, tag="cst", name="cst")
        nc.scalar.dma_start(cst, cst_d[:])
        ones_sb = wpool.tile([O, 1], fp16, tag="ones", name="ones")
        nc.scalar.dma_start(ones_sb, ones_d[:])
        b1_sb = {"u": cst[:, 0:2], "i": cst[:, 2:4]}
        b2_sb = {"u": cst[:O, 4:5], "i": cst[:O, 5:6]}

        w18_sb = {}
        w1b_sb = {}
        w2_sb = {}
        for name in ("u", "i"):
            w18_sb[name] = wpool.tile([128, K8T, H], fp8, tag=f"w18{name}", name=f"w18{name}")
            w1b_sb[name] = wpool.tile([128, KBT, H], bf16, tag=f"w1b{name}", name=f"w1b{name}")
            w2_sb[name] = wpool.tile([128, hh_t, O], fp16, tag=f"w2{name}", name=f"w2{name}")

        def l1_mms(sname, xt8, xtb):
            """Layer-1 matmuls (fp8 DoubleRow then bf16) for one tower."""
            ph = [ps_h.tile([128, nb], f32, tag="ph", name="ph") for _ in range(hh_t)]
            for k2 in range(0, K8T, 2):
                mv = xt8[:, k2 : k2 + 2, :]
                for hh in range(hh_t):
                    nc.tensor.matmul(
                        ph[hh],
                        w18_sb[sname][:, k2 : k2 + 2, hh * 128 : (hh + 1) * 128],
                        mv,
                        start=(k2 == 0),
                        stop=False,
                        perf_mode=DR,
                    )
            for k in range(KBT):
                mv = xtb[:, k, :]
                for hh in range(hh_t):
                    nc.tensor.matmul(
                        ph[hh],
                        w1b_sb[sname][:, k, hh * 128 : (hh + 1) * 128],
                        mv,
                        start=False,
                        stop=(k == KBT - 1),
                    )
            return ph

        def relu(sname, ph):
            hsb = [hp.tile([128, nb], fp16, tag="h", name="h") for _ in range(hh_t)]
            for hh in range(hh_t):
                nc.scalar.activation(
                    hsb[hh], ph[hh], AF.Relu,
                    bias=b1_sb[sname][:, hh : hh + 1], scale=1.0 / SW,
                )
            return hsb

        def tail_l2_dve(hs):
            """Layer 2 matmuls + DVE chain for a pending block; returns prod."""
            stash = {}
            for sname in ("u", "i"):
                puv = ps_uv.tile([O, nb], f32, tag="puv", name="puv")
                for hh in range(hh_t):
                    nc.tensor.matmul(
                        puv,
                        w2_sb[sname][:, hh, :],
                        hs[sname][hh],
                        start=(hh == 0),
                        stop=(hh == hh_t - 1),
                    )
                usb = uvp.tile([O, nb], fp16, tag="uv", name="uv")
                nc.vector.tensor_scalar_add(usb, puv, b2_sb[sname])
                stash[sname] = usb
            prod = uvp.tile([O, nb], fp16, tag="prod", name="prod")
            nc.vector.tensor_mul(prod, stash["u"], stash["i"])
            return prod

        def diag_sigmoid_out(r0, prod):
            ps = ps_d.tile([1, nb], f32, tag="psd", name="psd")
            nc.tensor.matmul(ps, ones_sb, prod, start=True, stop=True)
            s_blk = sp.tile([1, nb], f32, tag="sblk", name="s_blk")
            nc.scalar.activation(s_blk, ps, AF.Sigmoid)
            nc.scalar.dma_start(out[r0 : r0 + nb], s_blk)

        # All x/w transfers ride the single sync HWDGE ring in consumption
        # order (measured: splitting across rings desynchronizes delivery).
        def xdma(dst, src):
            nc.sync.dma_start(dst, src)

        def emit_block_dma(b, sname, xt8, xtb):
            if b == 0:
                # Interleave w1 and x chunks in PE consumption order with a
                # small leading chunk so the first matmuls start early.
                # (Measured: bulk data on the scalar HWDGE ring moves at
                # ~half the sync ring's rate — keep everything on sync.)
                for q0, q1 in ((0, 2), (2, 6), (6, K8T)):
                    xdma(w18_sb[sname][:, q0:q1, :], w18_d[sname][:, q0:q1, :])
                    xdma(xt8[:, q0:q1, :], x8_d[sname][:, b, q0:q1, :])
                hb = KBT // 2
                xdma(w1b_sb[sname][:, :hb, :], w1b_d[sname][:, :hb, :])
                xdma(xtb[:, :hb, :], xb_d[sname][:, b, :hb, :])
                xdma(w1b_sb[sname][:, hb:, :], w1b_d[sname][:, hb:, :])
                xdma(xtb[:, hb:, :], xb_d[sname][:, b, hb:, :])
            else:
                xdma(xt8, x8_d[sname][:, b, :, :])
                hb = KBT // 2
                xdma(xtb[:, :hb, :], xb_d[sname][:, b, :hb, :])
                xdma(xtb[:, hb:, :], xb_d[sname][:, b, hb:, :])

        # --- main loop ---
        # Emission order per block (see module docstring):
        #   DMAs | l1_mms(u) | tail(b-1) l2+DVE | relu(u) | l1_mms(i) with
        #   diag(b-1)+sigmoid deferred ~10 mms in | relu(i)
        pending = None  # (r0, hs dict)
        DIAG_DEFER = 10  # matmul instructions of the i-stream before diag

        def l1_mms_with_diag(sname, xt8, xtb, pend_diag):
            """Like l1_mms but emits pend_diag after DIAG_DEFER matmuls."""
            ph = [ps_h.tile([128, nb], f32, tag="ph", name="ph") for _ in range(hh_t)]
            n_mm = 0
            for k2 in range(0, K8T, 2):
                mv = xt8[:, k2 : k2 + 2, :]
                for hh in range(hh_t):
                    nc.tensor.matmul(
                        ph[hh],
                        w18_sb[sname][:, k2 : k2 + 2, hh * 128 : (hh + 1) * 128],
                        mv,
                        start=(k2 == 0),
                        stop=False,
                        perf_mode=DR,
                    )
                    n_mm += 1
                    if n_mm == DIAG_DEFER and pend_diag is not None:
                        diag_sigmoid_out(*pend_diag)
            for k in range(KBT):
                mv = xtb[:, k, :]
                for hh in range(hh_t):
                    nc.tensor.matmul(
                        ph[hh],
                        w1b_sb[sname][:, k, hh * 128 : (hh + 1) * 128],
                        mv,
                        start=False,
                        stop=(k == KBT - 1),
                    )
                    n_mm += 1
                    if n_mm == DIAG_DEFER and pend_diag is not None:
                        diag_sigmoid_out(*pend_diag)
            return ph

        for b in range(nblk):
            xt8s, xtbs = {}, {}
            for sname in ("u", "i"):
                xt8s[sname] = x8p.tile([128, K8T, nb], fp8, tag="xt8", name="xt8")
                xtbs[sname] = xbp.tile([128, KBT, nb], bf16, tag="xtb", name="xtb")
                emit_block_dma(b, sname, xt8s[sname], xtbs[sname])
            if b == 0:
                for sname in ("u", "i"):
                    xdma(w2_sb[sname], w2_d[sname][:])

            last = b == nblk - 1

            ph_u = l1_mms("u", xt8s["u"], xtbs["u"])
            prod_prev = None
            r0_prev = None
            if pending is not None:
                r0_prev, hs_prev = pending
                prod_prev = tail_l2_dve(hs_prev)
            hs_u = relu("u", ph_u)

            if not last:
                pend_diag = (r0_prev, prod_prev) if prod_prev is not None else None
                ph_i = l1_mms_with_diag("i", xt8s["i"], xtbs["i"], pend_diag)
                hs_i = relu("i", ph_i)
                pending = (b * nb, {"u": hs_u, "i": hs_i})
            else:
                # Last block: pull everything that can run early out of the
                # end-of-kernel serial chain.  u-tower layer 2 runs right
                # after tail(b-1); the i-tower runs h-sequentially with its
                # h0 layer-2 partial embedded mid-stream, so only
                # relu(h1) -> l2(h1) -> dot -> sigmoid remain at the end.
                puv_u = ps_uv.tile([O, nb], f32, tag="puv", name="puv")
                for hh in range(hh_t):
                    nc.tensor.matmul(
                        puv_u,
                        w2_sb["u"][:, hh, :],
                        hs_u[hh],
                        start=(hh == 0),
                        stop=(hh == hh_t - 1),
                    )
                usb_u = uvp.tile([O, nb], fp16, tag="uv", name="uv")
                nc.vector.tensor_scalar_add(usb_u, puv_u, b2_sb["u"])

                xt8, xtb = xt8s["i"], xtbs["i"]
                ph = [ps_h.tile([128, nb], f32, tag="ph", name="ph") for _ in range(hh_t)]
                hs_i = [hp.tile([128, nb], fp16, tag="h", name="h") for _ in range(hh_t)]
                puv_i = ps_uv.tile([O, nb], f32, tag="puv", name="puv")

                def i_col(hh, embed_l2_at=None, pend_diag=None):
                    n_mm = 0
                    for k2 in range(0, K8T, 2):
                        nc.tensor.matmul(
                            ph[hh],
                            w18_sb["i"][:, k2 : k2 + 2, hh * 128 : (hh + 1) * 128],
                            xt8[:, k2 : k2 + 2, :],
                            start=(k2 == 0), stop=False, perf_mode=DR,
                        )
                        n_mm += 1
                        if n_mm == DIAG_DEFER and pend_diag is not None:
                            diag_sigmoid_out(*pend_diag)
                        if embed_l2_at is not None and n_mm == embed_l2_at:
                            nc.tensor.matmul(
                                puv_i, w2_sb["i"][:, 0, :], hs_i[0],
                                start=True, stop=False,
                            )
                    for k in range(KBT):
                        nc.tensor.matmul(
                            ph[hh],
                            w1b_sb["i"][:, k, hh * 128 : (hh + 1) * 128],
                            xtb[:, k, :],
                            start=False, stop=(k == KBT - 1),
                        )
                        n_mm += 1
                        if n_mm == DIAG_DEFER and pend_diag is not None:
                            diag_sigmoid_out(*pend_diag)
                        if embed_l2_at is not None and n_mm == embed_l2_at:
                            nc.tensor.matmul(
                                puv_i, w2_sb["i"][:, 0, :], hs_i[0],
                                start=True, stop=False,
                            )

                pend_diag = ((b - 1) * nb, prod_prev) if prod_prev is not None else None
                i_col(0, pend_diag=pend_diag)
                nc.scalar.activation(
                    hs_i[0], ph[0], AF.Relu, bias=b1_sb["i"][:, 0:1], scale=1.0 / SW,
                )
                i_col(1, embed_l2_at=10)
                nc.scalar.activation(
                    hs_i[1], ph[1], AF.Relu, bias=b1_sb["i"][:, 1:2], scale=1.0 / SW,
                )
                nc.tensor.matmul(
                    puv_i, w2_sb["i"][:, 1, :], hs_i[1], start=False, stop=True
                )
                usb_i = uvp.tile([O, nb], fp16, tag="uv", name="uv")
                nc.vector.tensor_scalar_add(usb_i, puv_i, b2_sb["i"])
                prod = uvp.tile([O, nb], fp16, tag="prod", name="prod")
                nc.vector.tensor_mul(prod, usb_u, usb_i)
                diag_sigmoid_out(b * nb, prod)

    nc.compile()
    return nc


def _pack_cst(b1u, b1i, b2u, b2i):
    """[128, 6] f32: b1u as 2 cols, b1i as 2 cols, b2u, b2i (zero-padded)."""
    cst = np.zeros((128, 6), dtype=np.float32)
    cst[:, 0:2] = b1u.reshape(2, 128).T
    cst[:, 2:4] = b1i.reshape(2, 128).T
    cst[: b2u.shape[0], 4] = b2u
    cst[: b2i.shape[0], 5] = b2i
    return cst


def _pack_x(x_shard):
    """[Nc, D] f32 -> (fp8 [p, blk, K8T, n], bf16 [p, blk, KBT, n])."""
    nblk = x_shard.shape[0] // NB
    x4 = x_shard.reshape(nblk, NB, KT, 128)
    x8 = np.ascontiguousarray(
        x4[:, :, :K8T, :].astype(ml_dtypes.float8_e4m3).transpose(3, 0, 2, 1)
    )
    xb = np.ascontiguousarray(
        x4[:, :, K8T:, :].astype(ml_dtypes.bfloat16).transpose(3, 0, 2, 1)
    )
    return x8, xb


def _pack_w1(w):
    """[D, H] f32 -> (e4m3(SW*w) [p, K8T, H], bf16(SW*w) [p, KBT, H]).

    BOTH halves carry the SW pre-scale: the ReLU activation descales the
    whole PSUM by 1/SW, so every accumulated term must be scaled.  SW is a
    power of 2, so bf16(SW*w) == SW*bf16(w) exactly (exponent shift).
    """
    w4 = np.asarray(w, dtype=np.float32).reshape(KT, 128, w.shape[1])
    w8 = np.ascontiguousarray(
        (w4[:K8T] * np.float32(SW)).astype(ml_dtypes.float8_e4m3).transpose(1, 0, 2)
    )
    wb = np.ascontiguousarray(
        (w4[K8T:] * np.float32(SW)).astype(ml_dtypes.bfloat16).transpose(1, 0, 2)
    )
    return w8, wb


def _pack_w2(w):
    """[K*128, M] f32 -> fp16 [p, k, M] with (p,k,m) = w[k*128+p, m]."""
    kt = w.shape[0] // 128
    wb = np.asarray(w, dtype=np.float32).astype(np.float16)
    return np.ascontiguousarray(wb.reshape(kt, 128, w.shape[1]).transpose(1, 0, 2))


def _get_program():
    global _PROGRAM
    if _PROGRAM is None:
        _PROGRAM = _build_program()
    return _PROGRAM


def kernel(
    user_origin_emb,
    item_origin_emb,
    u_w1,
    u_b1,
    u_w2,
    u_b2,
    i_w1,
    i_b1,
    i_w2,
    i_b2,
):
    global LAST_RESULTS
    from concourse.bass_utils import run_bass_kernel_spmd

    xu = np.asarray(user_origin_emb, dtype=np.float32)
    xv = np.asarray(item_origin_emb, dtype=np.float32)
    ones = np.ones((O, 1), dtype=np.float16)
    w18u_, w1bu_ = _pack_w1(u_w1)
    w18i_, w1bi_ = _pack_w1(i_w1)
    shared = {
        "w18u": w18u_,
        "w1bu": w1bu_,
        "w18i": w18i_,
        "w1bi": w1bi_,
        "w2u": _pack_w2(u_w2),
        "w2i": _pack_w2(i_w2),
        "cst": _pack_cst(
            np.asarray(u_b1, dtype=np.float32),
            np.asarray(i_b1, dtype=np.float32),
            np.asarray(u_b2, dtype=np.float32),
            np.asarray(i_b2, dtype=np.float32),
        ),
        "ones": ones,
    }

    nc = _get_program()
    n_rows = xu.shape[0] // N_CORES
    in_maps = []
    for c in range(N_CORES):
        x8u_, xbu_ = _pack_x(xu[c * n_rows : (c + 1) * n_rows])
        x8v_, xbv_ = _pack_x(xv[c * n_rows : (c + 1) * n_rows])
        in_maps.append(
            {"x8u": x8u_, "xbu": xbu_, "x8v": x8v_, "xbv": xbv_, **shared}
        )
    res = run_bass_kernel_spmd(nc, in_maps, core_ids=list(range(N_CORES)), trace=TRACE)
    LAST_RESULTS = res
    return np.concatenate([r["out"] for r in res.results], axis=0)
